# revision 27
# baseline (speedup 1.0000x reference)
"""Trainium2 Bass kernel for BaseFisheyeLSSTransform (BEV pooling).

8-core SPMD, one program, per-core data tables.

- Host (sharding + index math): voxelize frustum geometry on jax-cpu;
  assign output rows (b, xrow) = "slots" to cores balanced by point
  count; shard x per core as its kept row-PAIRS (2 consecutive source
  rows, 80->128 ch padded, fp16, source-ascending order — a pure
  filter). Build the slot-major descriptor tables: each descriptor is
  one 512B pair; per-slot descriptor lists are quantized to 16 and
  chunked into 128-descriptor columns shared by all cores.
- Device: one InstDMAGatherAnt per 8 columns gathers 1024 512B pairs
  from the pair shard (int16 indices, single window). Per (column,
  lane) one DVE/Pool op builds M = (iota == y) * invcnt in fp16
  ([128, 360]); per (column, lane, slot-chunk-range) a partition-range
  matmul accumulates PSUM[slot] += pair_rows^T @ M. Closed slots are
  copied PSUM -> fp16 slab (Scalar engine) and flushed to DRAM.
- Host assembles [2, 80, 360, 360] fp32 from the 8 slabs.
"""
import sys

sys.path.insert(0, "/opt/trn_rl_repo")

import numpy as np

B, N, C = 2, 4, 80
FH, FW, D = 40, 60, 59
NX, NY = 360, 360
PB = N * D * FH * FW  # 566400 rows per batch
NROW = 2 * PB
NPAIR_SRC = NROW // 2
P = 128
Q = 16               # per-slot descriptor quantum (uniform across cores)
KCOL = 8             # columns (128 descs each) per dma_gather
FLUSH_WINDOWS = 16
POOL_FRAC = 0.3      # fraction of M-builds issued on gpsimd


def _ceil(a, b):
    return -(-a // b)


# ---------------------------------------------------------------- host side


def _geometry(camera2lidar_rots, camera2lidar_trans):
    import jax
    import jax.numpy as jnp

    cpu = jax.devices("cpu")[0]
    with jax.default_device(cpu):
        DX = jnp.array([0.3, 0.3, 8.0], dtype=jnp.float32)
        ORIGIN = jnp.array([-54.0, -54.0, -5.0], dtype=jnp.float32)
        ds = jnp.arange(1.0, 60.0, 1.0, dtype=jnp.float32)
        az = jnp.linspace(-1.92, 1.92, FW, dtype=jnp.float32)
        el = jnp.linspace(-0.61, 0.61, FH, dtype=jnp.float32)
        d_, e_, a_ = ds[:, None, None], el[None, :, None], az[None, None, :]
        xs = d_ * jnp.cos(e_) * jnp.sin(a_)
        ys = jnp.broadcast_to(d_ * jnp.sin(e_), (D, FH, FW))
        zs = d_ * jnp.cos(e_) * jnp.cos(a_)
        fr = jnp.stack([xs, ys, zs], axis=-1)
        geom = jnp.einsum("bnij,dhwj->bndhwi", camera2lidar_rots, fr)
        geom = geom + camera2lidar_trans[:, :, None, None, None, :]
        coords = np.asarray(((geom - ORIGIN) / DX).astype(jnp.int32))
    kept = (
        (coords[..., 0] >= 0) & (coords[..., 0] < NX)
        & (coords[..., 1] >= 0) & (coords[..., 1] < NY)
        & (coords[..., 2] >= 0) & (coords[..., 2] < 1)
    )
    return coords, kept


def build_schedule(camera2lidar_rots, camera2lidar_trans):
    coords, kept = _geometry(camera2lidar_rots, camera2lidar_trans)
    n_cores = 8

    gp = np.flatnonzero(kept.reshape(-1))          # kept flat rows
    cx = coords.reshape(-1, 3)[gp, 0]
    cy = coords.reshape(-1, 3)[gp, 1]
    bb = gp // PB
    lin = (bb * NX + cx) * NY + cy
    cnt = np.bincount(lin, minlength=B * NX * NY)
    invc = (1.0 / np.maximum(cnt, 1)).astype(np.float32)
    w_pt = invc[lin]

    # slots (b, xrow) balanced across cores by point count (greedy) so
    # per-slot-index sizes roughly match across cores (less padding)
    skey = bb * NX + cx
    s_cnt = np.bincount(skey, minlength=B * NX)
    order = np.argsort(-s_cnt, kind="stable")
    cores_slots = [[] for _ in range(n_cores)]
    load = [0] * n_cores
    for sk in order:
        if s_cnt[sk] == 0:
            continue
        ci = min(range(n_cores), key=lambda i: load[i])
        cores_slots[ci].append(int(sk))
        load[ci] += int(s_cnt[sk])
    NSLOTS = max(len(c) for c in cores_slots)

    slot_of_key = {}
    for ci in range(n_cores):
        for w, sk in enumerate(cores_slots[ci]):
            slot_of_key[(ci, sk)] = w

    # ---- per-core: kept pair list (source order) + per-slot descriptors
    per_core = []
    NPAIR_MAX = 0
    for ci in range(n_cores):
        own = np.isin(skey, np.array(cores_slots[ci], dtype=skey.dtype))
        g_ci = gp[own]
        w_ci = np.array([slot_of_key[(ci, int(k))] for k in skey[own]],
                        np.int32)
        y_ci = cy[own].astype(np.float32)
        inv_ci = w_pt[own]
        pairs = np.unique(g_ci // 2)               # source-ascending
        ppos = {int(pr): i for i, pr in enumerate(pairs)}
        NPAIR_MAX = max(NPAIR_MAX, len(pairs))
        pdescs = {}  # (slot, pair_idx) -> [(lane, y, inv)]
        for gpt, w_, y_, iv_ in zip(g_ci, w_ci, y_ci, inv_ci):
            pi = ppos[int(gpt) // 2]
            lane = int(gpt) % 2
            pdescs.setdefault((int(w_), pi), []).append(
                (lane, float(y_), float(iv_)))
        slot_descs = [[] for _ in range(NSLOTS)]
        for (w_, pi), lanes in sorted(pdescs.items()):
            slot_descs[w_].append((pi, lanes))
        per_core.append(dict(pairs=pairs, slot_descs=slot_descs))
    assert NPAIR_MAX <= 32767, f"pair shard too large: {NPAIR_MAX}"

    # ---- uniform per-slot chunk counts (quantum Q descs)
    nchunk = [0] * NSLOTS
    for w in range(NSLOTS):
        m = max(len(pc["slot_descs"][w]) for pc in per_core)
        nchunk[w] = max(_ceil(m, Q), 1)
    chunk_stream = []
    for w in range(NSLOTS):
        chunk_stream += [(w, q) for q in range(nchunk[w])]
    NCOL = _ceil(len(chunk_stream), P // Q)
    while len(chunk_stream) % (P // Q):
        chunk_stream.append((NSLOTS - 1, nchunk[NSLOTS - 1]))

    cols = []
    for c in range(NCOL):
        cols.append(chunk_stream[c * (P // Q): (c + 1) * (P // Q)])
    s2_instrs = []
    for c0 in range(0, NCOL, KCOL):
        s2_instrs.append(list(range(c0, min(c0 + KCOL, NCOL))))

    # segments per column: (lane, p0, p1, slot, vidcol, start, stop)
    # PE matmuls require partition base 0/32/64, so each segment gets its
    # own full-128 matmul with a dedicated vid column (-1 masks others)
    slot_first, slot_last = {}, {}
    seg_all = []
    NMB = 0
    for c in range(NCOL):
        segs = []
        for lane in range(2):
            groups = []
            for k, (w, q) in enumerate(cols[c]):
                if groups and groups[-1][0] == w and groups[-1][2] == k:
                    groups[-1][2] = k + 1
                else:
                    groups.append([w, k, k + 1])
            for (w, k0, k1) in groups:
                segs.append([lane, k0 * Q, k1 * Q, w, NMB])
                NMB += 1
        seg_all.append(segs)
    for c in range(NCOL):
        for si, seg in enumerate(seg_all[c]):
            w = seg[3]
            if w not in slot_first:
                slot_first[w] = (c, si)
            slot_last[w] = (c, si)
    for c in range(NCOL):
        for si, seg in enumerate(seg_all[c]):
            w = seg[3]
            seg.append(slot_first[w] == (c, si))
            seg.append(slot_last[w] == (c, si))

    copies_after = [[] for _ in range(NCOL)]
    for w, (c, si) in slot_last.items():
        copies_after[c].append(w)
    flushes = [[] for _ in range(NCOL)]
    nblocks = _ceil(NSLOTS, FLUSH_WINDOWS)
    for blk in range(nblocks):
        ws = [w for w in range(blk * FLUSH_WINDOWS,
                               min((blk + 1) * FLUSH_WINDOWS, NSLOTS))
              if w in slot_last]
        if ws:
            c = max(slot_last[w][0] for w in ws)
            flushes[c].append(blk)

    # ---- per-core tables: idx2 + vid/invpc
    for ci, pc in enumerate(per_core):
        slot_descs = pc["slot_descs"]
        vid = np.full((P, NMB), -1.0, np.float32)
        invpc = np.zeros((P, NMB), np.float32)
        idx2 = []
        for rec in s2_instrs:
            n = len(rec) * P
            arr = np.zeros(n, np.int16)
            for ki, c in enumerate(rec):
                segcol = {}
                for seg in seg_all[c]:
                    lane, p0, p1_, w, colv = seg[:5]
                    for k in range(p0 // Q, p1_ // Q):
                        segcol[(k, lane)] = colv
                for k, (w, q) in enumerate(cols[c]):
                    descs = slot_descs[w][q * Q: (q + 1) * Q]
                    for d, (pi, lanes) in enumerate(descs):
                        p = k * Q + d
                        arr[ki * P + p] = pi
                        for (lane, y_, iv_) in lanes:
                            colv = segcol[(k, lane)]
                            vid[p, colv] = y_
                            invpc[p, colv] = iv_
            idx2.append(arr)
        pc["idx2"] = idx2
        pc["vid"] = vid
        pc["invpc"] = invpc
        pc["slot_rows"] = [
            cores_slots[ci][w] if w < len(cores_slots[ci]) else None
            for w in range(NSLOTS)
        ]

    return dict(
        s2_instrs=s2_instrs, cols=cols, seg_all=seg_all,
        copies_after=copies_after, flushes=flushes,
        NSLOTS=NSLOTS, NCOL=NCOL, NMB=NMB, NPAIR_MAX=NPAIR_MAX,
        per_core=per_core, load=load, NINSTR=len(s2_instrs),
        NMBUILD=NMB,
    )


# ---------------------------------------------------------------- device


def _wrap16(arr):
    """int16 token list -> [128, ceil(n/16)] table (16-part blocks x8)."""
    n = len(arr)
    ncols = _ceil(n, 16)
    t = np.zeros((16, ncols), np.int16)
    for j in range(n):
        t[j % 16, j // 16] = arr[j]
    return np.tile(t, (8, 1))


def build_program(sched):
    import concourse.bacc as bacc
    import concourse.mybir as mybir
    import concourse.tile as tile
    from concourse.tile_rust import add_dep_helper

    f32, f16 = mybir.dt.float32, mybir.dt.float16
    i16 = mybir.dt.int16
    NSLOTS = sched["NSLOTS"]
    NMB = sched["NMB"]
    NPAIR = sched["NPAIR_MAX"]
    s2_instrs = sched["s2_instrs"]
    cols = sched["cols"]
    seg_all = sched["seg_all"]
    copies_after = sched["copies_after"]
    flushes = sched["flushes"]

    TOT2 = sum(len(r) * 8 for r in s2_instrs)

    nc = bacc.Bacc(None)
    xb = nc.declare_dram_parameter("xb", [NPAIR, 256], f16, isOutput=False)
    idx2_d = nc.declare_dram_parameter("idx2", [P, TOT2], i16,
                                       isOutput=False)
    vid_d = nc.declare_dram_parameter("vid", [P, NMB], f32, isOutput=False)
    invpc_d = nc.declare_dram_parameter("invpc", [P, NMB], f32,
                                        isOutput=False)
    iota_d = nc.declare_dram_parameter("iota", [P, NY], f16, isOutput=False)
    out_d = nc.declare_dram_parameter("out", [C, NSLOTS * NY], f16,
                                      isOutput=True)

    nseg = 0
    with tile.TileContext(nc) as tc:
        with (
            tc.tile_pool(name="const", bufs=1) as cpool,
            tc.tile_pool(name="g2", bufs=4) as g2pool,
            tc.tile_pool(name="m", bufs=12) as mpool,
            tc.tile_pool(name="psum", bufs=6, space="PSUM") as ppool,
            tc.tile_pool(name="hotp", bufs=1, space="PSUM") as hpool,
            tc.tile_pool(name="slab", bufs=3) as slabpool,
        ):
            idx2_t = cpool.tile([P, TOT2], i16)
            vid_t = cpool.tile([P, NMB], f32)
            invpc_t = cpool.tile([P, NMB], f32)
            iota_t = cpool.tile([P, NY], f16)
            nc.sync.dma_start(out=idx2_t[:], in_=idx2_d[:])
            nc.sync.dma_start(out=vid_t[:], in_=vid_d[:])
            nc.sync.dma_start(out=invpc_t[:], in_=invpc_d[:])
            nc.sync.dma_start(out=iota_t[:], in_=iota_d[:])

            s3sem = nc.alloc_semaphore()
            clr3 = nc.gpsimd.sem_clear(s3sem)

            # scratch PSUM tile kept warm with one tiny matmul per gather:
            # keeps the PE HAM clock-gate from throttling during stalls
            hot = hpool.tile([C, NY], f32, tag="hot", name="hotplate")

            wtiles = {}
            slabs = {}
            n2 = 0
            off2 = 0
            for rec in s2_instrs:
                ncols = len(rec)
                n = ncols * P
                g2 = g2pool.tile([P, ncols, 256], f16, tag="g2")
                gi2 = nc.gpsimd.dma_gather(
                    out_ap=g2[:],
                    in_ap=xb[:, :],
                    idxs_ap=idx2_t[:, off2: off2 + n // 16],
                    num_idxs=n,
                    num_idxs_reg=n,
                    elem_size=256,
                ).then_inc(s3sem, 16)
                add_dep_helper(gi2.ins, clr3.ins, reason="sem clear")
                n2 += 1
                off2 += n // 16
                wt = nc.tensor.wait_ge(s3sem, 16 * n2)
                add_dep_helper(wt.ins, gi2.ins, reason="issue order")
                nc.tensor.matmul(
                    hot[:], iota_t[:, :C], iota_t[:],
                    start=True, stop=True, skip_group_check=True,
                )
                for ki, c in enumerate(rec):
                    for seg in seg_all[c]:
                        lane, p0, p1_, w, colv, st, sp_ = seg
                        M = mpool.tile([P, NY], f16, tag="m")
                        eng = (nc.gpsimd
                               if int((nseg + 1) * POOL_FRAC)
                               > int(nseg * POOL_FRAC)
                               else nc.vector)
                        nseg += 1
                        eng.tensor_scalar(
                            out=M[:],
                            in0=iota_t[:],
                            scalar1=vid_t[:, colv: colv + 1],
                            scalar2=invpc_t[:, colv: colv + 1],
                            op0=mybir.AluOpType.is_equal,
                            op1=mybir.AluOpType.mult,
                        )
                        if st:
                            wtiles[w] = ppool.tile([C, NY], f32, tag="w",
                                                   name=f"w{w}")
                        mm = nc.tensor.matmul(
                            wtiles[w][:],
                            g2[:, ki, lane * 128: lane * 128 + 80],
                            M[:],
                            start=st,
                            stop=sp_,
                            skip_group_check=True,
                        )
                        add_dep_helper(mm.ins, wt.ins, reason="data ready")
                    for w in copies_after[c]:
                        blk = w // FLUSH_WINDOWS
                        if blk not in slabs:
                            slabs[blk] = slabpool.tile(
                                [C, FLUSH_WINDOWS * NY], f16, tag="slab",
                                name=f"slab{blk}",
                            )
                        off = w % FLUSH_WINDOWS
                        nc.scalar.activation(
                            out=slabs[blk][:, off * NY: (off + 1) * NY],
                            in_=wtiles.pop(w)[:],
                            func=mybir.ActivationFunctionType.Copy,
                        )
                    for blk in flushes[c]:
                        w0 = blk * FLUSH_WINDOWS
                        w1_ = min(w0 + FLUSH_WINDOWS, NSLOTS)
                        nc.sync.dma_start(
                            out=out_d[:, w0 * NY: w1_ * NY],
                            in_=slabs.pop(blk)[:, : (w1_ - w0) * NY],
                        )
    nc.compile()
    return nc


def make_in_maps(sched, x):
    xr = np.ascontiguousarray(x.reshape(NROW, C)).astype(np.float16)
    NPAIR = sched["NPAIR_MAX"]
    iota = np.broadcast_to(
        np.arange(NY, dtype=np.float16)[None, :], (P, NY)
    ).copy()
    in_maps = []
    for ci in range(8):
        pc = sched["per_core"][ci]
        pairs = pc["pairs"]
        shard = np.zeros((NPAIR, 2, 128), np.float16)
        shard[: len(pairs), :, :80] = xr.reshape(NPAIR_SRC, 2, C)[pairs]
        idx2 = np.concatenate([_wrap16(a) for a in pc["idx2"]], axis=1)
        in_maps.append(
            {
                "xb": shard.reshape(NPAIR, 256),
                "idx2": idx2,
                "vid": pc["vid"],
                "invpc": pc["invpc"],
                "iota": iota,
            }
        )
    return in_maps


def assemble(slabs, sched):
    out = np.zeros((B, C, NX, NY), np.float32)
    for ci in range(8):
        pc = sched["per_core"][ci]
        slab = np.asarray(slabs[ci], np.float32)
        for s, sk in enumerate(pc["slot_rows"]):
            if sk is None:
                continue
            b, xrow = sk // NX, sk % NX
            out[b, :, xrow, :] = slab[:, s * NY: (s + 1) * NY]
    return out


def kernel(x, camera2lidar_rots, camera2lidar_trans):
    from concourse.bass_utils import run_bass_kernel_spmd

    x = np.asarray(x, dtype=np.float32)
    rots = np.asarray(camera2lidar_rots, dtype=np.float32)
    trans = np.asarray(camera2lidar_trans, dtype=np.float32)
    sched = build_schedule(rots, trans)
    nc = build_program(sched)
    in_maps = make_in_maps(sched, x)
    res = run_bass_kernel_spmd(nc, in_maps, list(range(8)))
    slabs = [res.results[ci]["out"] for ci in range(8)]
    return assemble(slabs, sched)


# revision 28
# speedup vs baseline: 4.1796x; 4.1796x over previous
"""Trainium2 Bass kernel for BaseFisheyeLSSTransform (BEV pooling).

8-core SPMD, one program, per-core data tables.

- Host (sharding + index math): voxelize frustum geometry on jax-cpu;
  assign output rows (b, xrow) = "slots" to cores balanced by point
  count; shard x per core as its kept row-PAIRS (2 consecutive source
  rows, 80->128 ch padded, fp16, source-ascending order — a pure
  filter). Build the slot-major descriptor tables: each descriptor is
  one 512B pair; per-slot descriptor lists are quantized to 16 and
  chunked into 128-descriptor columns shared by all cores.
- Device: one InstDMAGatherAnt per 8 columns gathers 1024 512B pairs
  from the pair shard (int16 indices, single window). Per (column,
  lane) one DVE/Pool op builds M = (iota == y) * invcnt in fp16
  ([128, 360]); per (column, lane, slot-chunk-range) a partition-range
  matmul accumulates PSUM[slot] += pair_rows^T @ M. Closed slots are
  copied PSUM -> fp16 slab (Scalar engine) and flushed to DRAM.
- Host assembles [2, 80, 360, 360] fp32 from the 8 slabs.
"""
import sys

sys.path.insert(0, "/opt/trn_rl_repo")

import numpy as np

B, N, C = 2, 4, 80
FH, FW, D = 40, 60, 59
NX, NY = 360, 360
PB = N * D * FH * FW  # 566400 rows per batch
NROW = 2 * PB
NPAIR_SRC = NROW // 2
P = 128
Q = 16               # per-slot descriptor quantum (uniform across cores)
KCOL = 8             # columns (128 descs each) per dma_gather
FLUSH_WINDOWS = 16
POOL_FRAC = 0.0      # gpsimd tensor ops are slow Q7 ucode - keep M-builds on DVE


def _ceil(a, b):
    return -(-a // b)


# ---------------------------------------------------------------- host side


def _geometry(camera2lidar_rots, camera2lidar_trans):
    import jax
    import jax.numpy as jnp

    cpu = jax.devices("cpu")[0]
    with jax.default_device(cpu):
        DX = jnp.array([0.3, 0.3, 8.0], dtype=jnp.float32)
        ORIGIN = jnp.array([-54.0, -54.0, -5.0], dtype=jnp.float32)
        ds = jnp.arange(1.0, 60.0, 1.0, dtype=jnp.float32)
        az = jnp.linspace(-1.92, 1.92, FW, dtype=jnp.float32)
        el = jnp.linspace(-0.61, 0.61, FH, dtype=jnp.float32)
        d_, e_, a_ = ds[:, None, None], el[None, :, None], az[None, None, :]
        xs = d_ * jnp.cos(e_) * jnp.sin(a_)
        ys = jnp.broadcast_to(d_ * jnp.sin(e_), (D, FH, FW))
        zs = d_ * jnp.cos(e_) * jnp.cos(a_)
        fr = jnp.stack([xs, ys, zs], axis=-1)
        geom = jnp.einsum("bnij,dhwj->bndhwi", camera2lidar_rots, fr)
        geom = geom + camera2lidar_trans[:, :, None, None, None, :]
        coords = np.asarray(((geom - ORIGIN) / DX).astype(jnp.int32))
    kept = (
        (coords[..., 0] >= 0) & (coords[..., 0] < NX)
        & (coords[..., 1] >= 0) & (coords[..., 1] < NY)
        & (coords[..., 2] >= 0) & (coords[..., 2] < 1)
    )
    return coords, kept


def build_schedule(camera2lidar_rots, camera2lidar_trans):
    coords, kept = _geometry(camera2lidar_rots, camera2lidar_trans)
    n_cores = 8

    gp = np.flatnonzero(kept.reshape(-1))          # kept flat rows
    cx = coords.reshape(-1, 3)[gp, 0]
    cy = coords.reshape(-1, 3)[gp, 1]
    bb = gp // PB
    lin = (bb * NX + cx) * NY + cy
    cnt = np.bincount(lin, minlength=B * NX * NY)
    invc = (1.0 / np.maximum(cnt, 1)).astype(np.float32)
    w_pt = invc[lin]

    # slots (b, xrow) balanced across cores by point count (greedy) so
    # per-slot-index sizes roughly match across cores (less padding)
    skey = bb * NX + cx
    s_cnt = np.bincount(skey, minlength=B * NX)
    order = np.argsort(-s_cnt, kind="stable")
    cores_slots = [[] for _ in range(n_cores)]
    load = [0] * n_cores
    for sk in order:
        if s_cnt[sk] == 0:
            continue
        ci = min(range(n_cores), key=lambda i: load[i])
        cores_slots[ci].append(int(sk))
        load[ci] += int(s_cnt[sk])
    NSLOTS = max(len(c) for c in cores_slots)

    slot_of_key = {}
    for ci in range(n_cores):
        for w, sk in enumerate(cores_slots[ci]):
            slot_of_key[(ci, sk)] = w

    # ---- per-core: kept pair list (source order) + per-slot descriptors
    per_core = []
    NPAIR_MAX = 0
    for ci in range(n_cores):
        own = np.isin(skey, np.array(cores_slots[ci], dtype=skey.dtype))
        g_ci = gp[own]
        w_ci = np.array([slot_of_key[(ci, int(k))] for k in skey[own]],
                        np.int32)
        y_ci = cy[own].astype(np.float32)
        inv_ci = w_pt[own]
        pairs = np.unique(g_ci // 2)               # source-ascending
        ppos = {int(pr): i for i, pr in enumerate(pairs)}
        NPAIR_MAX = max(NPAIR_MAX, len(pairs))
        pdescs = {}  # (slot, pair_idx) -> [(lane, y, inv)]
        for gpt, w_, y_, iv_ in zip(g_ci, w_ci, y_ci, inv_ci):
            pi = ppos[int(gpt) // 2]
            lane = int(gpt) % 2
            pdescs.setdefault((int(w_), pi), []).append(
                (lane, float(y_), float(iv_)))
        slot_descs = [[] for _ in range(NSLOTS)]
        for (w_, pi), lanes in sorted(pdescs.items()):
            slot_descs[w_].append((pi, lanes))
        per_core.append(dict(pairs=pairs, slot_descs=slot_descs))
    assert NPAIR_MAX <= 32767, f"pair shard too large: {NPAIR_MAX}"

    # ---- uniform per-slot chunk counts (quantum Q descs)
    nchunk = [0] * NSLOTS
    for w in range(NSLOTS):
        m = max(len(pc["slot_descs"][w]) for pc in per_core)
        nchunk[w] = max(_ceil(m, Q), 1)
    chunk_stream = []
    for w in range(NSLOTS):
        chunk_stream += [(w, q) for q in range(nchunk[w])]
    NCOL = _ceil(len(chunk_stream), P // Q)
    while len(chunk_stream) % (P // Q):
        chunk_stream.append((NSLOTS - 1, nchunk[NSLOTS - 1]))

    cols = []
    for c in range(NCOL):
        cols.append(chunk_stream[c * (P // Q): (c + 1) * (P // Q)])
    s2_instrs = []
    for c0 in range(0, NCOL, KCOL):
        s2_instrs.append(list(range(c0, min(c0 + KCOL, NCOL))))

    # segments per column: (lane, p0, p1, slot, vidcol, start, stop)
    # PE matmuls require partition base 0/32/64, so each segment gets its
    # own full-128 matmul with a dedicated vid column (-1 masks others)
    slot_first, slot_last = {}, {}
    seg_all = []
    NMB = 0
    for c in range(NCOL):
        segs = []
        for lane in range(2):
            groups = []
            for k, (w, q) in enumerate(cols[c]):
                if groups and groups[-1][0] == w and groups[-1][2] == k:
                    groups[-1][2] = k + 1
                else:
                    groups.append([w, k, k + 1])
            for (w, k0, k1) in groups:
                segs.append([lane, k0 * Q, k1 * Q, w, NMB])
                NMB += 1
        seg_all.append(segs)
    for c in range(NCOL):
        for si, seg in enumerate(seg_all[c]):
            w = seg[3]
            if w not in slot_first:
                slot_first[w] = (c, si)
            slot_last[w] = (c, si)
    for c in range(NCOL):
        for si, seg in enumerate(seg_all[c]):
            w = seg[3]
            seg.append(slot_first[w] == (c, si))
            seg.append(slot_last[w] == (c, si))

    copies_after = [[] for _ in range(NCOL)]
    for w, (c, si) in slot_last.items():
        copies_after[c].append(w)
    flushes = [[] for _ in range(NCOL)]
    nblocks = _ceil(NSLOTS, FLUSH_WINDOWS)
    for blk in range(nblocks):
        ws = [w for w in range(blk * FLUSH_WINDOWS,
                               min((blk + 1) * FLUSH_WINDOWS, NSLOTS))
              if w in slot_last]
        if ws:
            c = max(slot_last[w][0] for w in ws)
            flushes[c].append(blk)

    # ---- per-core tables: idx2 + vid/invpc
    for ci, pc in enumerate(per_core):
        slot_descs = pc["slot_descs"]
        vid = np.full((P, NMB), -1.0, np.float32)
        invpc = np.zeros((P, NMB), np.float32)
        idx2 = []
        for rec in s2_instrs:
            n = len(rec) * P
            arr = np.zeros(n, np.int16)
            for ki, c in enumerate(rec):
                segcol = {}
                for seg in seg_all[c]:
                    lane, p0, p1_, w, colv = seg[:5]
                    for k in range(p0 // Q, p1_ // Q):
                        segcol[(k, lane)] = colv
                for k, (w, q) in enumerate(cols[c]):
                    descs = slot_descs[w][q * Q: (q + 1) * Q]
                    for d, (pi, lanes) in enumerate(descs):
                        p = k * Q + d
                        arr[ki * P + p] = pi
                        for (lane, y_, iv_) in lanes:
                            colv = segcol[(k, lane)]
                            vid[p, colv] = y_
                            invpc[p, colv] = iv_
            idx2.append(arr)
        pc["idx2"] = idx2
        pc["vid"] = vid
        pc["invpc"] = invpc
        pc["slot_rows"] = [
            cores_slots[ci][w] if w < len(cores_slots[ci]) else None
            for w in range(NSLOTS)
        ]

    return dict(
        s2_instrs=s2_instrs, cols=cols, seg_all=seg_all,
        copies_after=copies_after, flushes=flushes,
        NSLOTS=NSLOTS, NCOL=NCOL, NMB=NMB, NPAIR_MAX=NPAIR_MAX,
        per_core=per_core, load=load, NINSTR=len(s2_instrs),
        NMBUILD=NMB,
    )


# ---------------------------------------------------------------- device


def _wrap16(arr):
    """int16 token list -> [128, ceil(n/16)] table (16-part blocks x8)."""
    n = len(arr)
    ncols = _ceil(n, 16)
    t = np.zeros((16, ncols), np.int16)
    for j in range(n):
        t[j % 16, j // 16] = arr[j]
    return np.tile(t, (8, 1))


def build_program(sched):
    import concourse.bacc as bacc
    import concourse.mybir as mybir
    import concourse.tile as tile
    from concourse.tile_rust import add_dep_helper

    f32, f16 = mybir.dt.float32, mybir.dt.float16
    i16 = mybir.dt.int16
    NSLOTS = sched["NSLOTS"]
    NMB = sched["NMB"]
    NPAIR = sched["NPAIR_MAX"]
    s2_instrs = sched["s2_instrs"]
    cols = sched["cols"]
    seg_all = sched["seg_all"]
    copies_after = sched["copies_after"]
    flushes = sched["flushes"]

    TOT2 = sum(len(r) * 8 for r in s2_instrs)

    nc = bacc.Bacc(None)
    xb = nc.declare_dram_parameter("xb", [NPAIR, 256], f16, isOutput=False)
    idx2_d = nc.declare_dram_parameter("idx2", [P, TOT2], i16,
                                       isOutput=False)
    vid_d = nc.declare_dram_parameter("vid", [P, NMB], f32, isOutput=False)
    invpc_d = nc.declare_dram_parameter("invpc", [P, NMB], f32,
                                        isOutput=False)
    iota_d = nc.declare_dram_parameter("iota", [P, NY], f16, isOutput=False)
    out_d = nc.declare_dram_parameter("out", [C, NSLOTS * NY], f16,
                                      isOutput=True)

    nseg = 0
    with tile.TileContext(nc) as tc:
        with (
            tc.tile_pool(name="const", bufs=1) as cpool,
            tc.tile_pool(name="g2", bufs=4) as g2pool,
            tc.tile_pool(name="m", bufs=12) as mpool,
            tc.tile_pool(name="psum", bufs=6, space="PSUM") as ppool,
            tc.tile_pool(name="hotp", bufs=1, space="PSUM") as hpool,
            tc.tile_pool(name="slab", bufs=3) as slabpool,
        ):
            idx2_t = cpool.tile([P, TOT2], i16)
            vid_t = cpool.tile([P, NMB], f32)
            invpc_t = cpool.tile([P, NMB], f32)
            iota_t = cpool.tile([P, NY], f16)
            nc.sync.dma_start(out=idx2_t[:], in_=idx2_d[:])
            nc.sync.dma_start(out=vid_t[:], in_=vid_d[:])
            nc.sync.dma_start(out=invpc_t[:], in_=invpc_d[:])
            nc.sync.dma_start(out=iota_t[:], in_=iota_d[:])

            s3sem = nc.alloc_semaphore()
            clr3 = nc.gpsimd.sem_clear(s3sem)

            # scratch PSUM tile kept warm with one tiny matmul per gather:
            # keeps the PE HAM clock-gate from throttling during stalls
            hot = hpool.tile([C, NY], f32, tag="hot", name="hotplate")

            wtiles = {}
            slabs = {}
            n2 = 0
            off2 = 0
            for rec in s2_instrs:
                ncols = len(rec)
                n = ncols * P
                g2 = g2pool.tile([P, ncols, 256], f16, tag="g2")
                gi2 = nc.gpsimd.dma_gather(
                    out_ap=g2[:],
                    in_ap=xb[:, :],
                    idxs_ap=idx2_t[:, off2: off2 + n // 16],
                    num_idxs=n,
                    num_idxs_reg=n,
                    elem_size=256,
                ).then_inc(s3sem, 16)
                add_dep_helper(gi2.ins, clr3.ins, reason="sem clear")
                n2 += 1
                off2 += n // 16
                wt = nc.tensor.wait_ge(s3sem, 16 * n2)
                add_dep_helper(wt.ins, gi2.ins, reason="issue order")
                nc.tensor.matmul(
                    hot[:], iota_t[:, :C], iota_t[:],
                    start=True, stop=True, skip_group_check=True,
                )
                for ki, c in enumerate(rec):
                    for seg in seg_all[c]:
                        lane, p0, p1_, w, colv, st, sp_ = seg
                        M = mpool.tile([P, NY], f16, tag="m")
                        eng = (nc.gpsimd
                               if int((nseg + 1) * POOL_FRAC)
                               > int(nseg * POOL_FRAC)
                               else nc.vector)
                        nseg += 1
                        eng.tensor_scalar(
                            out=M[:],
                            in0=iota_t[:],
                            scalar1=vid_t[:, colv: colv + 1],
                            scalar2=invpc_t[:, colv: colv + 1],
                            op0=mybir.AluOpType.is_equal,
                            op1=mybir.AluOpType.mult,
                        )
                        if st:
                            wtiles[w] = ppool.tile([C, NY], f32, tag="w",
                                                   name=f"w{w}")
                        mm = nc.tensor.matmul(
                            wtiles[w][:],
                            g2[:, ki, lane * 128: lane * 128 + 80],
                            M[:],
                            start=st,
                            stop=sp_,
                            skip_group_check=True,
                        )
                        add_dep_helper(mm.ins, wt.ins, reason="data ready")
                    for w in copies_after[c]:
                        blk = w // FLUSH_WINDOWS
                        if blk not in slabs:
                            slabs[blk] = slabpool.tile(
                                [C, FLUSH_WINDOWS * NY], f16, tag="slab",
                                name=f"slab{blk}",
                            )
                        off = w % FLUSH_WINDOWS
                        nc.scalar.activation(
                            out=slabs[blk][:, off * NY: (off + 1) * NY],
                            in_=wtiles.pop(w)[:],
                            func=mybir.ActivationFunctionType.Copy,
                        )
                    for blk in flushes[c]:
                        w0 = blk * FLUSH_WINDOWS
                        w1_ = min(w0 + FLUSH_WINDOWS, NSLOTS)
                        nc.sync.dma_start(
                            out=out_d[:, w0 * NY: w1_ * NY],
                            in_=slabs.pop(blk)[:, : (w1_ - w0) * NY],
                        )
    nc.compile()
    return nc


def make_in_maps(sched, x):
    xr = np.ascontiguousarray(x.reshape(NROW, C)).astype(np.float16)
    NPAIR = sched["NPAIR_MAX"]
    iota = np.broadcast_to(
        np.arange(NY, dtype=np.float16)[None, :], (P, NY)
    ).copy()
    in_maps = []
    for ci in range(8):
        pc = sched["per_core"][ci]
        pairs = pc["pairs"]
        shard = np.zeros((NPAIR, 2, 128), np.float16)
        shard[: len(pairs), :, :80] = xr.reshape(NPAIR_SRC, 2, C)[pairs]
        idx2 = np.concatenate([_wrap16(a) for a in pc["idx2"]], axis=1)
        in_maps.append(
            {
                "xb": shard.reshape(NPAIR, 256),
                "idx2": idx2,
                "vid": pc["vid"],
                "invpc": pc["invpc"],
                "iota": iota,
            }
        )
    return in_maps


def assemble(slabs, sched):
    out = np.zeros((B, C, NX, NY), np.float32)
    for ci in range(8):
        pc = sched["per_core"][ci]
        slab = np.asarray(slabs[ci], np.float32)
        for s, sk in enumerate(pc["slot_rows"]):
            if sk is None:
                continue
            b, xrow = sk // NX, sk % NX
            out[b, :, xrow, :] = slab[:, s * NY: (s + 1) * NY]
    return out


def kernel(x, camera2lidar_rots, camera2lidar_trans):
    from concourse.bass_utils import run_bass_kernel_spmd

    x = np.asarray(x, dtype=np.float32)
    rots = np.asarray(camera2lidar_rots, dtype=np.float32)
    trans = np.asarray(camera2lidar_trans, dtype=np.float32)
    sched = build_schedule(rots, trans)
    nc = build_program(sched)
    in_maps = make_in_maps(sched, x)
    res = run_bass_kernel_spmd(nc, in_maps, list(range(8)))
    slabs = [res.results[ci]["out"] for ci in range(8)]
    return assemble(slabs, sched)
